# revision 1
# baseline (speedup 1.0000x reference)
"""Single-head attention with QKV projections for TRN2, batch-sharded across
8 NeuronCores (one batch element per core).

Reference computation per batch element (S=2048, D=1024, fp32):
    Q = xq @ Wq + bq ; K = xk @ Wk + bk ; V = xv @ Wv + bv
    L = Q @ K^T                      # [S, S]
    out = (softmax(L, -1) * 1/sqrt(D)) @ V

Per-core plan (all matmuls fp32r = full-rate fp32 on the PE; empirically
~310ns per 128x128x512 matmul incl. the serial weight load):
  Phase A-Q: xq tiles -> PE-transpose (f32r) -> xq^T ; Q^T = Wq^T @ xq^T
             -> DRAM scratch [D, S]
  Phase A-K: K^T = Wk^T @ xk^T -> resident SBUF [D, S]
  Phase A-V: V = xv @ Wv       -> resident SBUF [S, D]
  Phase B (per 512-col q strip of Q^T):
    L^T tiles [sk=128, sq=512] = K^T_tile.T @ Q^T-strip   (PSUM)
    U^T = exp(L^T) on ACT straight out of PSUM (no max subtraction: |L| < ~80
      so exp stays inside fp32 range), written as f32r
    rowsum[1, sq] += ones[128,1].T @ U^T_t   (PE, accumulated over sk tiles)
    rowsumT[sq-tile, 1] = rowsum_slice.T via K=1 matmul; recipT = (1/32)/rowsumT
    out[sq, d] = U^T_slice.T @ V  accumulated over sk tiles, normalized by
      per-partition recipT via DVE tensor_scalar, + bv broadcast.
"""
import numpy as np
from contextlib import ExitStack

import concourse.bass as bass
import concourse.bacc as bacc
import concourse.tile as tile
import concourse.mybir as mybir
from concourse.bass_utils import run_bass_kernel_spmd

F32 = mybir.dt.float32
F32R = mybir.dt.float32r
AF = mybir.ActivationFunctionType

B, S, D = 8, 2048, 1024
NKT = D // 128          # 8 contraction tiles
NST = S // 128          # 16 s tiles
SCALE = 1.0 / 32.0      # 1/sqrt(D)

_CACHED = {}


def build(nrep=1, barrier=False):
    nc = bacc.Bacc("TRN2", target_bir_lowering=False, debug=False, num_devices=8)

    xq = nc.dram_tensor("xq", [S, D], F32R, kind="ExternalInput")
    xk = nc.dram_tensor("xk", [S, D], F32R, kind="ExternalInput")
    xv = nc.dram_tensor("xv", [S, D], F32R, kind="ExternalInput")
    wq = nc.dram_tensor("wq", [D, D], F32R, kind="ExternalInput")
    wk = nc.dram_tensor("wk", [D, D], F32R, kind="ExternalInput")
    wv = nc.dram_tensor("wv", [D, D], F32R, kind="ExternalInput")
    bqd = nc.dram_tensor("bqd", [128, NKT], F32, kind="ExternalInput")  # bq.reshape(8,128).T
    bkd = nc.dram_tensor("bkd", [128, NKT], F32, kind="ExternalInput")
    bvd = nc.dram_tensor("bvd", [1, D], F32R, kind="ExternalInput")
    identd = nc.dram_tensor("identd", [128, 128], F32R, kind="ExternalInput")
    ones1d = nc.dram_tensor("ones1d", [1, 128], F32R, kind="ExternalInput")
    onespd = nc.dram_tensor("onespd", [128, 1], F32R, kind="ExternalInput")

    out = nc.dram_tensor("out", [S, D], F32, kind="ExternalOutput")
    qt_dram = nc.dram_tensor("qt_scratch", [D, S], F32R)  # internal scratch

    with tile.TileContext(nc) as tc, ExitStack() as ctx:
        # ---------------- persistent pools ----------------
        cpool = ctx.enter_context(tc.tile_pool(name="const", bufs=1))
        ktp = ctx.enter_context(tc.tile_pool(name="ktr", bufs=1))
        vsp = ctx.enter_context(tc.tile_pool(name="vres", bufs=1))
        pp = ctx.enter_context(tc.tile_pool(name="pp", bufs=3, space="PSUM"))
        op = ctx.enter_context(tc.tile_pool(name="op", bufs=3, space="PSUM"))

        ident = cpool.tile([128, 128], F32R, tag="ident")
        bqs = cpool.tile([128, NKT], F32, tag="bqs")
        bks = cpool.tile([128, NKT], F32, tag="bks")
        bvs = cpool.tile([1, D], F32R, tag="bvs")
        ones1 = cpool.tile([1, 128], F32R, tag="ones1")
        onesp = cpool.tile([128, 1], F32R, tag="onesp")
        bvb = cpool.tile([128, D], F32, tag="bvb")
        nc.gpsimd.dma_start(ident[:], identd.ap())
        nc.gpsimd.dma_start(bqs[:], bqd.ap())
        nc.gpsimd.dma_start(bks[:], bkd.ap())
        nc.gpsimd.dma_start(bvs[:], bvd.ap())
        nc.gpsimd.dma_start(ones1[:], ones1d.ap())
        nc.gpsimd.dma_start(onesp[:], onespd.ap())

        # broadcast bv across partitions via K=1 matmul: bvb = ones1.T @ bvs
        for h in range(2):
            bps = op.tile([128, 512], F32, tag="av")
            nc.tensor.matmul(bps[:], ones1[:], bvs[:, h * 512:(h + 1) * 512],
                             start=True, stop=True)
            nc.scalar.copy(bvb[:, h * 512:(h + 1) * 512], bps[:])

        # ---------------- phase A: projections ----------------
        def load_w(wpool, w_dram):
            w_s = wpool.tile([128, NKT * D], F32R, tag="w")
            for k in range(NKT):
                nc.gpsimd.dma_start(w_s[:, k * D:(k + 1) * D],
                                  w_dram.ap()[k * 128:(k + 1) * 128, :])
            return w_s

        def transpose_strip(tp, xpool, xtpool, x_dram, j, n_stiles):
            """Load x rows [j*128*n .. ) and produce x^T strip [D, 128*n] (f32r)."""
            xt = xtpool.tile([128, NKT * 128 * n_stiles], F32R, tag="xt")
            for st in range(n_stiles):
                xl = xpool.tile([128, D], F32R, tag="xl")
                nc.sync.dma_start(
                    xl[:], x_dram.ap()[(j * n_stiles + st) * 128:(j * n_stiles + st + 1) * 128, :])
                for k4 in range(NKT // 4):
                    tpt = tp.tile([128, 512], F32R, tag="tp")
                    for kk in range(4):
                        k = k4 * 4 + kk
                        nc.tensor.transpose(tpt[:, kk * 128:(kk + 1) * 128],
                                            xl[:, k * 128:(k + 1) * 128], ident[:])
                    # scatter 4 transposed tiles into xt at (k, st) slots
                    dst = xt[:].rearrange("p (k s) -> p k s", s=128 * n_stiles)
                    nc.vector.tensor_copy(
                        dst[:, k4 * 4:k4 * 4 + 4, st * 128:(st + 1) * 128], tpt[:])
            return xt

        for _rep in range(nrep):
          if _rep and barrier:
              tc.strict_bb_all_engine_barrier()
          with ExitStack() as actx:
            tp = actx.enter_context(tc.tile_pool(name="tp", bufs=2, space="PSUM"))
            wpool = actx.enter_context(tc.tile_pool(name="wpool", bufs=1))
            xpool = actx.enter_context(tc.tile_pool(name="xpool", bufs=2))
            xtpool = actx.enter_context(tc.tile_pool(name="xtpool", bufs=1))

            # ---- A-Q: Q^T -> DRAM scratch ----
            with nc.named_scope("phase_aq"), ExitStack() as qctx:
                qstg = qctx.enter_context(tc.tile_pool(name="qstg", bufs=3))
                w_s = load_w(wpool, wq)
                for j in range(4):
                    xt = transpose_strip(tp, xpool, xtpool, xq, j, 4)
                    for m in range(NKT):
                        ppt = pp.tile([128, 512], F32, tag="pp")
                        for k in range(NKT):
                            nc.tensor.matmul(
                                ppt[:],
                                w_s[:, k * D + m * 128:k * D + (m + 1) * 128],
                                xt[:, k * 512:(k + 1) * 512],
                                start=(k == 0), stop=(k == NKT - 1))
                        qs_t = qstg.tile([128, 512], F32R, tag="qs")
                        nc.scalar.activation(qs_t[:], ppt[:], AF.Identity,
                                             bias=bqs[:, m:m + 1])
                        nc.scalar.dma_start(
                            qt_dram.ap()[m * 128:(m + 1) * 128, j * 512:(j + 1) * 512],
                            qs_t[:])

            # ---- A-K: K^T resident ----
            kt = ktp.tile([128, NKT * S], F32R, tag="kt")       # K^T resident
            with nc.named_scope("phase_ak"):
                w_s = load_w(wpool, wk)
                for j in range(4):
                    xt = transpose_strip(tp, xpool, xtpool, xk, j, 4)
                    for m in range(NKT):
                        ppt = pp.tile([128, 512], F32, tag="pp")
                        for k in range(NKT):
                            nc.tensor.matmul(
                                ppt[:],
                                w_s[:, k * D + m * 128:k * D + (m + 1) * 128],
                                xt[:, k * 512:(k + 1) * 512],
                                start=(k == 0), stop=(k == NKT - 1))
                        nc.scalar.activation(
                            kt[:, m * S + j * 512:m * S + (j + 1) * 512],
                            ppt[:], AF.Identity, bias=bks[:, m:m + 1])

            # ---- A-V: V resident ----
            vs = vsp.tile([128, NST * D], F32R, tag="vs")       # V resident
            with nc.named_scope("phase_av"):
                w_s = load_w(wpool, wv)
                for j in range(4):
                    xt = transpose_strip(tp, xpool, xtpool, xv, j, 4)
                    for m in range(4):          # s tiles within strip
                        sg = j * 4 + m
                        for h in range(2):      # dout halves
                            ppt = pp.tile([128, 512], F32, tag="pp")
                            for k in range(NKT):
                                nc.tensor.matmul(
                                    ppt[:],
                                    xt[:, k * 512 + m * 128:k * 512 + (m + 1) * 128],
                                    w_s[:, k * D + h * 512:k * D + (h + 1) * 512],
                                    start=(k == 0), stop=(k == NKT - 1))
                            nc.vector.tensor_copy(
                                vs[:, sg * D + h * 512:sg * D + (h + 1) * 512], ppt[:])

        # ---------------- phase B: attention (transposed logits) ----------------
          with ExitStack() as bctx, nc.named_scope("phase_b"):
            qsp = bctx.enter_context(tc.tile_pool(name="qsp", bufs=1))
            utp = bctx.enter_context(tc.tile_pool(name="utp", bufs=1))
            osp = bctx.enter_context(tc.tile_pool(name="osp", bufs=2))
            rsp = bctx.enter_context(tc.tile_pool(name="rsp", bufs=2))
            rsps = bctx.enter_context(tc.tile_pool(name="rsps", bufs=1, space="PSUM"))
            rtps = bctx.enter_context(tc.tile_pool(name="rtps", bufs=1, space="PSUM"))

            for j in range(4):                  # q strips of 512
                qs = qsp.tile([128, NKT * 512], F32R, tag="qs")
                src = qt_dram.ap()[:, j * 512:(j + 1) * 512]
                nc.sync.dma_start(
                    qs[:].rearrange("p (k s) -> p k s", s=512),
                    src.rearrange("(k p) s -> p k s", p=128))

                # L^T tiles + exp -> U^T strip [S, 512] (f32r)
                ut = utp.tile([128, NST * 512], F32R, tag="ut")
                for t in range(NST):
                    lpt = pp.tile([128, 512], F32, tag="pp")
                    for k in range(NKT):
                        nc.tensor.matmul(
                            lpt[:],
                            kt[:, k * S + t * 128:k * S + (t + 1) * 128],
                            qs[:, k * 512:(k + 1) * 512],
                            start=(k == 0), stop=(k == NKT - 1))
                    nc.scalar.activation(ut[:, t * 512:(t + 1) * 512],
                                         lpt[:], AF.Exp)

                # rowsum over sk (partition dim) via ones matmuls -> [1, 512]
                rs_ps = rsps.tile([1, 512], F32, tag="rs")
                for t in range(NST):
                    nc.tensor.matmul(rs_ps[:], onesp[:],
                                     ut[:, t * 512:(t + 1) * 512],
                                     start=(t == 0), stop=(t == NST - 1))
                rs_sb = rsp.tile([1, 512], F32R, tag="rssb")
                nc.scalar.copy(rs_sb[:], rs_ps[:])

                for m in range(4):              # q tiles of 128 within strip
                    sq = j * 4 + m
                    # rowsumT [128,1] via K=1 matmul, then recipT = (1/32)/rowsum
                    rt_ps = rtps.tile([128, 2], F32, tag="rt")
                    nc.tensor.matmul(rt_ps[:],
                                     rs_sb[:, m * 128:(m + 1) * 128],
                                     ones1[:, 0:2], start=True, stop=True)
                    rct = rsp.tile([128, 1], F32, tag="rct")
                    nc.vector.reciprocal(rct[:], rt_ps[:, 0:1])
                    nc.vector.tensor_scalar_mul(rct[:], rct[:], SCALE)

                    # out[sq, :] = U^T_slice.T @ V, normalized + bv
                    os_t = osp.tile([128, D], F32, tag="os")
                    for h in range(2):
                        opt = op.tile([128, 512], F32, tag="av")
                        for t in range(NST):
                            nc.tensor.matmul(
                                opt[:],
                                ut[:, t * 512 + m * 128:t * 512 + (m + 1) * 128],
                                vs[:, t * D + h * 512:t * D + (h + 1) * 512],
                                start=(t == 0), stop=(t == NST - 1))
                        nc.vector.tensor_scalar_mul(
                            os_t[:, h * 512:(h + 1) * 512], opt[:], rct[:])
                    nc.vector.tensor_add(os_t[:], os_t[:], bvb[:])
                    nc.scalar.dma_start(out.ap()[sq * 128:(sq + 1) * 128, :], os_t[:])

    nc.compile()
    return nc


def _get_nc():
    if "nc" not in _CACHED:
        _CACHED["nc"] = build()
    return _CACHED["nc"]


def make_in_maps(q, k, v, Wq, bq, Wk, bk, Wv, bv):
    q = np.ascontiguousarray(q, np.float32)
    k = np.ascontiguousarray(k, np.float32)
    v = np.ascontiguousarray(v, np.float32)
    consts = {
        "wq": np.ascontiguousarray(Wq, np.float32),
        "wk": np.ascontiguousarray(Wk, np.float32),
        "wv": np.ascontiguousarray(Wv, np.float32),
        "bqd": np.ascontiguousarray(np.asarray(bq, np.float32).reshape(NKT, 128).T),
        "bkd": np.ascontiguousarray(np.asarray(bk, np.float32).reshape(NKT, 128).T),
        "bvd": np.asarray(bv, np.float32).reshape(1, D).copy(),
        "identd": np.eye(128, dtype=np.float32),
        "ones1d": np.ones((1, 128), np.float32),
        "onespd": np.ones((128, 1), np.float32),
    }
    return [dict(consts, xq=q[c], xk=k[c], xv=v[c]) for c in range(B)]


def kernel(q, k, v, Wq, bq, Wk, bk, Wv, bv, _trace=False, _trace_kwargs=None):
    in_maps = make_in_maps(q, k, v, Wq, bq, Wk, bk, Wv, bv)
    nc = _get_nc()
    res = run_bass_kernel_spmd(nc, in_maps, core_ids=list(range(B)),
                               trace=_trace, **(_trace_kwargs or {}))
    out = np.stack([res.results[c]["out"] for c in range(B)])
    if _trace:
        kernel.last_results = res
    return out



# revision 8
# speedup vs baseline: 6.1112x; 6.1112x over previous
"""Single-head attention with QKV projections for TRN2, batch-sharded across
8 NeuronCores (one batch element per core).

Reference computation per batch element (S=2048, D=1024, fp32):
    Q = xq @ Wq + bq ; K = xk @ Wk + bk ; V = xv @ Wv + bv
    L = Q @ K^T                      # [S, S]
    out = (softmax(L, -1) * 1/sqrt(D)) @ V

v4 plan (fused, no DRAM scratch, stall-free feeds):
  Phase K: stream xk -> PE transpose -> K^T = Wk^T @ xk^T resident [D,S] f32r.
  Phase V: stream xv -> V = xv @ Wv + bv resident [S,D] bf16.
  Phase B per 512-wide q strip: Q^T strip fused, then
    L^T tiles = K^T_tile.T @ Q^T-strip; U^T = exp(L^T) -> bf16 on ACT;
    P += U^T_t on Pool; rowsum via one ones-matmul; rct = (1/32)/rowsum;
    out = U^T.T @ V (bf16), normalized on ACT via per-partition scale=rct.
  Feeds: projections run as two 256-wide half-passes so transposes (and
  their xl DMAs) interleave with matmuls; W streams as 16 half-tiles with
  the halves needed first loaded first, split across two HWDGE queues.

PSUM (8 banks x 2KB): ppA-D (1 buf) + av (2) + tp (2) = 16KB exactly.
"""
import numpy as np
from contextlib import ExitStack

import concourse.bass as bass
import concourse.bacc as bacc
import concourse.tile as tile
import concourse.mybir as mybir
from concourse.bass_utils import run_bass_kernel_spmd

F32 = mybir.dt.float32
F32R = mybir.dt.float32r
BF16 = mybir.dt.bfloat16
AF = mybir.ActivationFunctionType

B, S, D = 8, 2048, 1024
NKT = D // 128          # 8 contraction tiles
NST = S // 128          # 16 s tiles
SCALE = 1.0 / 32.0      # 1/sqrt(D)

_CACHED = {}


def build(nrep=1, barrier=False):
    nc = bacc.Bacc("TRN2", target_bir_lowering=False, debug=False, num_devices=8)

    xq = nc.dram_tensor("xq", [S, D], F32R, kind="ExternalInput")
    xk = nc.dram_tensor("xk", [S, D], F32R, kind="ExternalInput")
    xv = nc.dram_tensor("xv", [S, D], F32R, kind="ExternalInput")
    wq = nc.dram_tensor("wq", [D, D], F32R, kind="ExternalInput")
    wk = nc.dram_tensor("wk", [D, D], F32R, kind="ExternalInput")
    wv = nc.dram_tensor("wv", [D, D], F32R, kind="ExternalInput")
    bqd = nc.dram_tensor("bqd", [128, NKT], F32, kind="ExternalInput")  # bq.reshape(8,128).T
    bkd = nc.dram_tensor("bkd", [128, NKT], F32, kind="ExternalInput")
    bvd = nc.dram_tensor("bvd", [1, D], F32, kind="ExternalInput")
    identd = nc.dram_tensor("identd", [128, 128], F32R, kind="ExternalInput")
    ones1d = nc.dram_tensor("ones1d", [1, 128], F32R, kind="ExternalInput")
    onespd = nc.dram_tensor("onespd", [128, 1], F32R, kind="ExternalInput")

    out = nc.dram_tensor("out", [S, D], F32, kind="ExternalOutput")

    with tile.TileContext(nc) as tc, ExitStack() as ctx:
        # ---------------- pools ----------------
        cpool = ctx.enter_context(tc.tile_pool(name="const", bufs=1))
        ktp = ctx.enter_context(tc.tile_pool(name="ktr", bufs=1))
        vsp = ctx.enter_context(tc.tile_pool(name="vres", bufs=1))
        wpool = ctx.enter_context(tc.tile_pool(name="wpool", bufs=1))
        xpool = ctx.enter_context(tc.tile_pool(name="xpool", bufs=2))
        xtpool = ctx.enter_context(tc.tile_pool(name="xtpool", bufs=1))
        qsp = ctx.enter_context(tc.tile_pool(name="qsp", bufs=1))
        utp = ctx.enter_context(tc.tile_pool(name="utp", bufs=1))
        osp = ctx.enter_context(tc.tile_pool(name="osp", bufs=2))
        accp = ctx.enter_context(tc.tile_pool(name="accp", bufs=1))
        rsp = ctx.enter_context(tc.tile_pool(name="rsp", bufs=1))
        ps = ctx.enter_context(tc.tile_pool(name="ps", bufs=1, space="PSUM"))

        ident = cpool.tile([128, 128], F32R, tag="ident")
        bqs = cpool.tile([128, NKT], F32, tag="bqs")
        bks = cpool.tile([128, NKT], F32, tag="bks")
        bvs = cpool.tile([1, D], F32, tag="bvs")
        ones1 = cpool.tile([1, 128], F32R, tag="ones1")
        onesp = cpool.tile([128, 1], F32R, tag="onesp")
        bvb = cpool.tile([128, D], F32, tag="bvb")
        nc.gpsimd.dma_start(ident[:], identd.ap())
        nc.gpsimd.dma_start(bqs[:], bqd.ap())
        nc.gpsimd.dma_start(bks[:], bkd.ap())
        nc.gpsimd.dma_start(bvs[:], bvd.ap())
        nc.gpsimd.dma_start(ones1[:], ones1d.ap())
        nc.gpsimd.dma_start(onesp[:], onespd.ap())
        nc.gpsimd.partition_broadcast(bvb[:], bvs[:], channels=128)

        def load_w(w_dram):
            """16 half-tile DMAs, h0 halves first (they are consumed first),
            split across the vector and scalar HWDGE queues."""
            w_s = wpool.tile([128, NKT * D], F32R, tag="w")
            for h, eng in ((0, nc.scalar), (1, nc.scalar)):
                for k in range(NKT):
                    eng.dma_start(
                        w_s[:, k * D + h * 512:k * D + (h + 1) * 512],
                        w_dram.ap()[k * 128:(k + 1) * 128, h * 512:(h + 1) * 512])
            return w_s

        def half_transposes(x_dram, j, xt, half):
            """Transpose x rows [j*512+half*256, +256) into xt columns
            [half*256, +256) of each k block."""
            for st in (0, 1) if half == 0 else (2, 3):
                xl = xpool.tile([128, D], F32R, tag="xl")
                nc.sync.dma_start(
                    xl[:], x_dram.ap()[(j * 4 + st) * 128:(j * 4 + st + 1) * 128, :])
                for k4 in range(NKT // 4):
                    tpt = ps.tile([128, 512], F32R, tag="tp", bufs=2)
                    for kk in range(4):
                        k = k4 * 4 + kk
                        nc.tensor.transpose(tpt[:, kk * 128:(kk + 1) * 128],
                                            xl[:, k * 128:(k + 1) * 128], ident[:])
                    dst = xt[:].rearrange("p (k s) -> p k s", s=512)
                    nc.vector.tensor_copy(
                        dst[:, k4 * 4:k4 * 4 + 4, st * 128:(st + 1) * 128], tpt[:])

        PTAGS = ("ppA", "ppB", "ppC", "ppD")

        def proj_strip(x_dram, j, w_s, write_cb):
            """out[m] = sum_k W[k,m]^T @ x^T[k, strip] for m in 0..7.
            Emission: [T_A][mh0 colsA][T_B][mh0 colsB][wb 0-3][mh1 colsA]
            [mh1 colsB][wb 4-7] — strip halves interleave with transposes,
            m-halves consume the W h0 half before the h1 half arrives."""
            xt = xtpool.tile([128, NKT * 512], F32R, tag="xt")
            half_transposes(x_dram, j, xt, 0)
            for mh in range(2):
                pps = [ps.tile([128, 512], F32, tag=PTAGS[i], name=f"pp{i}")
                       for i in range(4)]
                for sh in range(2):
                    if mh == 0 and sh == 1:
                        half_transposes(x_dram, j, xt, 1)
                    for k in range(NKT):
                        for i in range(4):
                            m = mh * 4 + i
                            nc.tensor.matmul(
                                pps[i][:, sh * 256:(sh + 1) * 256],
                                w_s[:, k * D + m * 128:k * D + (m + 1) * 128],
                                xt[:, k * 512 + sh * 256:k * 512 + sh * 256 + 256],
                                start=(k == 0), stop=(k == NKT - 1))
                for i in range(4):
                    write_cb(mh * 4 + i, pps[i])

        for _rep in range(nrep):
          if _rep and barrier:
              tc.strict_bb_all_engine_barrier()

          # ---------------- phase K ----------------
          kt = ktp.tile([128, NKT * S], F32R, tag="kt")       # K^T resident
          with nc.named_scope("phase_k"):
            w_s = load_w(wk)
            for j in range(4):
                def wr_k(m, ppt, j=j):
                    nc.scalar.activation(
                        kt[:, m * S + j * 512:m * S + (j + 1) * 512],
                        ppt[:], AF.Identity, bias=bks[:, m:m + 1])
                proj_strip(xk, j, w_s, wr_k)

          # ---------------- phase V ----------------
          vs = vsp.tile([128, NST * D], BF16, tag="vs")       # V resident (bf16)
          with nc.named_scope("phase_v"):
            w_s = load_w(wv)
            for j in range(4):
                xt = xtpool.tile([128, NKT * 512], F32R, tag="xt")

                def vpass(sts, h, tags, j=j, xt=xt):
                    pps = [ps.tile([128, 512], F32, tag=tags[i], name=f"vp{i}")
                           for i in range(2)]
                    for k in range(NKT):
                        for i, stt in enumerate(sts):
                            nc.tensor.matmul(
                                pps[i][:],
                                xt[:, k * 512 + stt * 128:k * 512 + (stt + 1) * 128],
                                w_s[:, k * D + h * 512:k * D + (h + 1) * 512],
                                start=(k == 0), stop=(k == NKT - 1))
                    for i, stt in enumerate(sts):
                        sg = j * 4 + stt
                        nc.vector.tensor_add(
                            vs[:, sg * D + h * 512:sg * D + (h + 1) * 512],
                            pps[i][:], bvb[:, h * 512:(h + 1) * 512])

                half_transposes(xv, j, xt, 0)
                vpass((0, 1), 0, ("ppA", "ppB"))
                half_transposes(xv, j, xt, 1)
                vpass((2, 3), 0, ("ppC", "ppD"))
                vpass((0, 1), 1, ("ppA", "ppB"))
                vpass((2, 3), 1, ("ppC", "ppD"))

          # ---------------- phase B: fused Q-proj + attention ----------------
          with nc.named_scope("phase_b"):
            w_s = load_w(wq)
            for j in range(4):                  # q strips of 512
                qs = qsp.tile([128, NKT * 512], F32R, tag="qs")

                def wr_q(m, ppt, qs=qs):
                    nc.scalar.activation(qs[:, m * 512:(m + 1) * 512],
                                         ppt[:], AF.Identity,
                                         bias=bqs[:, m:m + 1])
                proj_strip(xq, j, w_s, wr_q)

                # L^T tiles + exp -> U^T strip [S, 512] bf16; P += U^T_t (Pool)
                ut = utp.tile([128, NST * 512], BF16, tag="ut")
                pacc = accp.tile([128, 512], F32R, tag="pacc")
                for t in range(NST):
                    lpt = ps.tile([128, 512], F32, tag=PTAGS[t % 2], name="lpt")
                    for k in range(NKT):
                        nc.tensor.matmul(
                            lpt[:],
                            kt[:, k * S + t * 128:k * S + (t + 1) * 128],
                            qs[:, k * 512:(k + 1) * 512],
                            start=(k == 0), stop=(k == NKT - 1))
                    nc.scalar.activation(ut[:, t * 512:(t + 1) * 512],
                                         lpt[:], AF.Exp)
                    if t == 0:
                        nc.gpsimd.tensor_copy(pacc[:], ut[:, 0:512])
                    else:
                        nc.gpsimd.tensor_add(pacc[:], pacc[:],
                                             ut[:, t * 512:(t + 1) * 512])

                def av_mms(m, h):
                    opt = ps.tile([128, 512], F32, tag="av", bufs=2)
                    for t in range(NST):
                        nc.tensor.matmul(
                            opt[:],
                            ut[:, t * 512 + m * 128:t * 512 + (m + 1) * 128],
                            vs[:, t * D + h * 512:t * D + (h + 1) * 512],
                            start=(t == 0), stop=(t == NST - 1))
                    return opt

                def make_rct(m, rs_sb):
                    rt_ps = ps.tile([128, 2], F32, tag="ppD", name="rt_ps")
                    nc.tensor.matmul(rt_ps[:],
                                     rs_sb[:, m * 128:(m + 1) * 128],
                                     ones1[:, 0:2], start=True, stop=True)
                    rct = rsp.tile([128, 1], F32, tag=f"rct{m}", name="rct")
                    nc.vector.reciprocal(rct[:], rt_ps[:, 0:1])
                    nc.vector.tensor_scalar_mul(rct[:], rct[:], SCALE)
                    return rct

                def norm_store(m, h, opt, rct, os_t):
                    nc.scalar.activation(os_t[:, h * 512:(h + 1) * 512],
                                         opt[:], AF.Identity, scale=rct[:])
                    sq = j * 4 + m
                    nc.scalar.dma_start(
                        out.ap()[sq * 128:(sq + 1) * 128, h * 512:(h + 1) * 512],
                        os_t[:, h * 512:(h + 1) * 512])

                # AV(m0,h0) first so the rowsum finale never stalls the PE;
                # rowsum + rct0 resolve while AV(m0,h1) runs.
                os0 = osp.tile([128, D], F32, tag="os")
                opt00 = av_mms(0, 0)
                rs_ps = ps.tile([1, 512], F32, tag="ppC", name="rs_ps")
                nc.tensor.matmul(rs_ps[:], onesp[:], pacc[:],
                                 start=True, stop=True)
                rs_sb = rsp.tile([1, 512], F32R, tag="rssb")
                nc.scalar.copy(rs_sb[:], rs_ps[:])
                rct0 = make_rct(0, rs_sb)
                norm_store(0, 0, opt00, rct0, os0)
                opt01 = av_mms(0, 1)
                rcts = [rct0] + [make_rct(m, rs_sb) for m in range(1, 4)]
                norm_store(0, 1, opt01, rct0, os0)
                for m in range(1, 4):
                    os_m = osp.tile([128, D], F32, tag="os", name="os_m")
                    for h in range(2):
                        opt = av_mms(m, h)
                        norm_store(m, h, opt, rcts[m], os_m)

    nc.compile()
    return nc


def _get_nc():
    if "nc" not in _CACHED:
        _CACHED["nc"] = build()
    return _CACHED["nc"]


def make_in_maps(q, k, v, Wq, bq, Wk, bk, Wv, bv):
    q = np.ascontiguousarray(q, np.float32)
    k = np.ascontiguousarray(k, np.float32)
    v = np.ascontiguousarray(v, np.float32)
    consts = {
        "wq": np.ascontiguousarray(Wq, np.float32),
        "wk": np.ascontiguousarray(Wk, np.float32),
        "wv": np.ascontiguousarray(Wv, np.float32),
        "bqd": np.ascontiguousarray(np.asarray(bq, np.float32).reshape(NKT, 128).T),
        "bkd": np.ascontiguousarray(np.asarray(bk, np.float32).reshape(NKT, 128).T),
        "bvd": np.asarray(bv, np.float32).reshape(1, D).copy(),
        "identd": np.eye(128, dtype=np.float32),
        "ones1d": np.ones((1, 128), np.float32),
        "onespd": np.ones((128, 1), np.float32),
    }
    return [dict(consts, xq=q[c], xk=k[c], xv=v[c]) for c in range(B)]


def kernel(q, k, v, Wq, bq, Wk, bk, Wv, bv, _trace=False, _trace_kwargs=None):
    in_maps = make_in_maps(q, k, v, Wq, bq, Wk, bk, Wv, bv)
    nc = _get_nc()
    res = run_bass_kernel_spmd(nc, in_maps, core_ids=list(range(B)),
                               trace=_trace, **(_trace_kwargs or {}))
    out = np.stack([res.results[c]["out"] for c in range(B)])
    if _trace:
        kernel.last_results = res
    return out
